# revision 1
# baseline (speedup 1.0000x reference)
"""BitLinear (ternary-weight linear) Trainium2 kernel.

Math (matching the reference):
    s      = max(act_scale, 1e-5)
    x_int  = clip(round(x / s), -127, 127)          # int8-valued
    out    = (x_int * s) @ ((packed_w - 1) * alpha).T + bias
           = (alpha * s) * (x_int @ sign(W).T) + bias

Device strategy (8 cores, data-parallel over the 16384 tokens):
  - Each core gets 2048 tokens of x (f32), plus replicated W^T in bf16
    (values in {-1, 0, +1} -> exact in bf16) and a partition-replicated bias.
  - On-chip quantization produces x_int exactly as bf16 integers
    (|x_int| <= 127 -> exact in bf16). The matmul x_int @ signW^T is then
    exact integer arithmetic in the f32 PSUM accumulator (partial sums
    < 2^24), so the only rounding vs the reference is the final
    (alpha*s) scaling - relative error ~1e-7.
  - The hardware f32->int conversion rounds to nearest-even, exactly
    matching jnp.round, so quantization is: ACT scales by 1/s and
    converts f32->int16 (RNE), then one DVE op clamps to [-127, 127]
    and converts int16->bf16. (CoreSim truncates here instead - known
    sim-vs-HW divergence; hardware is truth.)
  - x_int (bf16, token-major) is bounced through a DRAM scratch tensor so
    the xbar DMA-transpose can reload it with the contract dim (IN) on
    partitions for the matmul, in a few large efficient transfers.
"""

import sys

sys.path.insert(0, "/opt/trn_rl_repo")

import numpy as np
import ml_dtypes

# ---- problem constants (hardcoded per harness contract) ----
B, S, IN, OUT = 4, 4096, 4096, 4096
TOKENS = B * S              # 16384
N_CORES = 8
T = TOKENS // N_CORES       # 2048 tokens per core
HALF = T // 2               # (kept for probe scripts)
Q = T // 4                  # 512-token quarter processed per inner pass
KT = IN // 128              # 32 k-tiles (contraction)
N_CHUNK = 512               # output columns per PSUM tile
NT = OUT // N_CHUNK         # 8 n-chunks
MT = Q // 128               # 4 m-tiles (tokens) per quarter
XCH = 2048                  # free-dim chunk for quantization staging


def _build_program(inv_s: float, alpha_s: float, reps: int = 1):
    import concourse.mybir as mybir
    import concourse.tile as tile
    from concourse import bacc

    nc = bacc.Bacc("TRN2", target_bir_lowering=False, debug=False,
                   num_devices=N_CORES)

    x_d = nc.dram_tensor("x", [T, IN], mybir.dt.float32, kind="ExternalInput")
    # wt[p, k, o] = sign(W)^T[k*128 + p, o]
    wt_d = nc.dram_tensor("wt", [128, KT, OUT], mybir.dt.bfloat16,
                          kind="ExternalInput")
    bias_d = nc.dram_tensor("bias", [128, OUT], mybir.dt.float32,
                            kind="ExternalInput")
    out_d = nc.dram_tensor("out", [T, OUT], mybir.dt.float32,
                           kind="ExternalOutput")
    xq_d = nc.dram_tensor("xq_scratch", [T, IN], mybir.dt.bfloat16)

    AF = mybir.ActivationFunctionType
    ALU = mybir.AluOpType

    with tile.TileContext(nc) as tc:
        with (
            tc.tile_pool(name="stage", bufs=2) as stage,
            tc.tile_pool(name="xqt", bufs=2) as xqt_pool,
            tc.tile_pool(name="wtp", bufs=2) as wt_pool,
            tc.tile_pool(name="outsb", bufs=4) as out_pool,
            tc.tile_pool(name="biasp", bufs=1) as bias_pool,
            tc.tile_pool(name="psum", bufs=8, space="PSUM") as psum_pool,
        ):
            bias_t = bias_pool.tile([128, OUT], mybir.dt.float32, tag="bias")
            nc.scalar.dma_start(bias_t[:], bias_d.ap())

            # DMA ring assignment: SP (nc.sync) carries only the xbar
            # transposes so their WAR waits never head-of-line-block other
            # loads; gpsimd (SWDGE) carries x loads; ACT (nc.scalar)
            # carries xq writes, weight loads, bias and output stores.
            def emit_quant(quarter, c):
                """Quantize one 128-token row-chunk: x -> round/clip -> bf16,
                bounce to DRAM."""
                r0 = quarter * Q + c * 128
                for qq in range(IN // XCH):
                    i0 = qq * XCH
                    xt = stage.tile([128, XCH], mybir.dt.float32, tag="xf32")
                    nc.gpsimd.dma_start(xt[:],
                                        x_d.ap()[r0:r0 + 128, i0:i0 + XCH])
                    t2 = stage.tile([128, XCH], mybir.dt.int16, tag="t2")
                    nc.scalar.activation(t2[:], xt[:], AF.Copy,
                                         bias=0.0, scale=float(inv_s))
                    qb = stage.tile([128, XCH], mybir.dt.bfloat16, tag="qb")
                    nc.vector.tensor_scalar(qb[:], t2[:], 127.0, -127.0,
                                            ALU.min, ALU.max)
                    nc.scalar.dma_start(xq_d.ap()[r0:r0 + 128, i0:i0 + XCH],
                                        qb[:])

            def emit_transpose(quarter, xqT):
                t0 = quarter * Q
                for k in range(KT):
                    nc.sync.dma_start_transpose(
                        xqT[:, k * Q:(k + 1) * Q],
                        xq_d.ap()[t0:t0 + Q, k * 128:(k + 1) * 128])

            def load_wt(n):
                wt = wt_pool.tile([128, KT * N_CHUNK], mybir.dt.bfloat16,
                                  tag="wt")
                nc.scalar.dma_start(
                    wt[:].rearrange("p (k o) -> p k o", k=KT),
                    wt_d.ap()[:, :, n * N_CHUNK:(n + 1) * N_CHUNK])
                return wt

            def emit_nchunk(quarter, n, xqT, wt=None):
                t0 = quarter * Q
                if wt is None:
                    wt = load_wt(n)
                for m in range(MT):
                    psum = psum_pool.tile([128, N_CHUNK], mybir.dt.float32)
                    for k in range(KT):
                        nc.tensor.matmul(
                            psum[:],
                            xqT[:, k * Q + m * 128:k * Q + (m + 1) * 128],
                            wt[:, k * N_CHUNK:(k + 1) * N_CHUNK],
                            start=(k == 0), stop=(k == KT - 1))
                    osb = out_pool.tile([128, N_CHUNK], mybir.dt.float32,
                                        tag="osb")
                    nc.scalar.activation(osb[:], psum[:], AF.Copy,
                                         bias=0.0, scale=float(alpha_s))
                    nc.vector.tensor_tensor(
                        osb[:], osb[:],
                        bias_t[:, n * N_CHUNK:(n + 1) * N_CHUNK], ALU.add)
                    nc.scalar.dma_start(
                        out_d.ap()[t0 + m * 128:t0 + (m + 1) * 128,
                                   n * N_CHUNK:(n + 1) * N_CHUNK],
                        osb[:])

            CH = Q // 128               # row-chunks per quarter (4)
            NQ = T // Q                 # quarters (4)
            for _rep in range(reps):
                # prefetch the first weight chunk while phase A runs
                wt0 = load_wt(0)
                # lead-in: quantize quarters 0,1; transpose quarter 0
                for c in range(CH):
                    emit_quant(0, c)
                for c in range(CH):
                    emit_quant(1, c)
                xqTs = [None] * NQ
                xqT_first = xqt_pool.tile([128, KT * Q], mybir.dt.bfloat16,
                                          tag="xqT")
                xqTs[0] = xqT_first
                emit_transpose(0, xqTs[0])
                for q in range(NQ):
                    # next quarter's transposes run (on the dedicated SP
                    # ring) while this quarter's matmuls stream
                    if q + 1 < NQ:
                        xqT_next = xqt_pool.tile(
                            [128, KT * Q], mybir.dt.bfloat16, tag="xqT")
                        xqTs[q + 1] = xqT_next
                        emit_transpose(q + 1, xqTs[q + 1])
                    for n in range(NT):
                        emit_nchunk(q, n, xqTs[q],
                                    wt=wt0 if (q == 0 and n == 0) else None)
                        # quantize quarters 2,3 under the first two
                        # quarters' matmul streams
                        if q < 2 and n < CH:
                            emit_quant(q + 2, n)

    nc.compile()
    return nc


def kernel(x, packed_w, alpha, act_scale, bias, _trace=False):
    from concourse.bass_utils import run_bass_kernel_spmd

    x2d = np.asarray(x, dtype=np.float32).reshape(TOKENS, IN)
    s = max(float(np.asarray(act_scale)), 1e-5)
    inv_s = 1.0 / np.float32(s)
    alpha_s = float(np.float32(np.asarray(alpha, dtype=np.float32)) *
                    np.float32(s))

    # sign weights {-1,0,1} -> bf16 exact; layout wt[p, k, o] = W^T[k*128+p, o]
    w_sign = (np.asarray(packed_w, dtype=np.float32) - 1.0)          # [OUT, IN]
    wT = w_sign.T.astype(ml_dtypes.bfloat16)                         # [IN, OUT]
    whost = np.ascontiguousarray(
        wT.reshape(KT, 128, OUT).transpose(1, 0, 2))                 # [128,KT,OUT]
    bias_rep = np.ascontiguousarray(
        np.broadcast_to(np.asarray(bias, dtype=np.float32)[None, :],
                        (128, OUT)))                                 # [128, OUT]

    nc = _build_program(float(inv_s), alpha_s)

    in_maps = [
        {"x": np.ascontiguousarray(x2d[c * T:(c + 1) * T]),
         "wt": whost, "bias": bias_rep}
        for c in range(N_CORES)
    ]
    res = run_bass_kernel_spmd(nc, in_maps, list(range(N_CORES)),
                               trace=_trace)

    out = np.empty((TOKENS, OUT), dtype=np.float32)
    for c in range(N_CORES):
        out[c * T:(c + 1) * T] = res.results[c]["out"]
    out = out.reshape(B, S, OUT)
    if _trace:
        return out, res
    return out



# revision 2
# speedup vs baseline: 1.2999x; 1.2999x over previous
"""BitLinear (ternary-weight linear) Trainium2 kernel — fp8 DoubleRow version.

Math (matching the reference):
    s      = max(act_scale, 1e-5)
    z      = clip(round(x / s), -127, 127)           # int8-valued
    out    = (alpha * s) * (z @ sign(W).T) + bias

Key idea: TRN2's fp8 DoubleRow matmul contracts 2 k-tiles (256 deep) per
instruction at 0.5 cycles per output row -> 4x bf16 MAC throughput. z in
[-127,127] is not exact in fp8e4 (4 sig bits), so split exactly:
    h   = round(z / 16)        (any rounding mode works)
    h16 = 16 * h               in {-128..128, step 16}  -> exact in fp8e4
    l   = z - h16              in [-15, 15] integers    -> exact in fp8e4
    z @ W = h16 @ W + l @ W    (both passes accumulate into the same PSUM
                                bank; f32 accumulator keeps integer
                                arithmetic exact)
Two DoubleRow passes = 2x bf16 matmul throughput overall: ~437us of PE
time per core vs the bf16 baseline's ~874us.

Device strategy (8 cores, data-parallel over the 16384 tokens, 2048 each):
  - Quantize x token-major (ACT round+scale, DVE clamp) -> z int16, bounce
    through DRAM, xbar-DMA-transpose (2-byte granularity) back k-major.
  - Convert zT -> h16/l fp8 tiles on ACT/DVE/Pool (layout- and
    rounding-mode-robust: any h with |z-16h|<=15 recombines exactly).
  - Weights replicated, host-packed to fp8e4 [128, NT, KT, 512]; streamed
    once (16MB) as one n-chunk at a time, double buffered.
  - PSUM drain fused on DVE: out = psum * (alpha*s) + bias -> bf16 store
    (bf16 output rounding ~1.3e-3 rel, well within 2e-2; host upcasts).
"""

import sys

sys.path.insert(0, "/opt/trn_rl_repo")

import numpy as np
import ml_dtypes

# ---- problem constants (hardcoded per harness contract) ----
B, S, IN, OUT = 4, 4096, 4096, 4096
TOKENS = B * S              # 16384
N_CORES = 8
T = TOKENS // N_CORES       # 2048 tokens per core
KT = IN // 128              # 32 k-tiles (contraction)
KP = KT // 2                # 16 DoubleRow k-pairs
N_CHUNK = 512               # output columns per PSUM tile
NT = OUT // N_CHUNK         # 8 n-chunks
Q = 512                     # token-quarter (transpose/convert granularity)
NQ = T // Q                 # 4 quarters
MT = Q // 128               # 4 m-tiles per quarter
XCH = 2048                  # free-dim chunk for quantization staging


def _build_program(inv_s: float, alpha_s: float, reps: int = 1):
    import concourse.mybir as mybir
    import concourse.tile as tile
    from concourse import bacc

    nc = bacc.Bacc("TRN2", target_bir_lowering=False, debug=False,
                   num_devices=N_CORES)

    x_d = nc.dram_tensor("x", [T, IN], mybir.dt.float32, kind="ExternalInput")
    # wt[p, n, k, c] = sign(W)[n*512 + c, k*128 + p]
    wt_d = nc.dram_tensor("wt", [128, NT, KT, N_CHUNK], mybir.dt.float8e4,
                          kind="ExternalInput")
    bias_d = nc.dram_tensor("bias", [128, OUT], mybir.dt.bfloat16,
                            kind="ExternalInput")
    out_d = nc.dram_tensor("out", [T, OUT], mybir.dt.bfloat16,
                           kind="ExternalOutput")
    z_d = nc.dram_tensor("z_scratch", [T, IN], mybir.dt.int16)

    AF = mybir.ActivationFunctionType
    ALU = mybir.AluOpType
    DR = mybir.MatmulPerfMode.DoubleRow

    with tile.TileContext(nc) as tc:
        with (
            tc.tile_pool(name="xstage", bufs=2) as xstage,
            tc.tile_pool(name="zstage", bufs=2) as zstage,
            tc.tile_pool(name="ztp", bufs=4) as ztp,
            tc.tile_pool(name="hp", bufs=4) as hp,
            tc.tile_pool(name="hl", bufs=2 * NQ) as hl_pool,
            tc.tile_pool(name="wtp", bufs=2) as wt_pool,
            tc.tile_pool(name="outsb", bufs=4) as out_pool,
            tc.tile_pool(name="biasp", bufs=1) as bias_pool,
            tc.tile_pool(name="psum", bufs=8, space="PSUM") as psum_pool,
        ):
            bias_t = bias_pool.tile([128, OUT], mybir.dt.bfloat16, tag="bias")
            nc.scalar.dma_start(bias_t[:], bias_d.ap())

            # DMA ring assignment: SP (nc.sync) carries the xbar transposes;
            # Pool/SWDGE (nc.gpsimd) carries x loads; ACT (nc.scalar)
            # carries z stores, weight loads, bias load and output stores.
            def emit_quant(m):
                """Quantize one 128-token row block: x -> round/clip ->
                int16, bounce to DRAM."""
                r0 = m * 128
                for c in range(IN // XCH):
                    i0 = c * XCH
                    xt = xstage.tile([128, XCH], mybir.dt.float32, tag="xf32")
                    nc.gpsimd.dma_start(xt[:],
                                        x_d.ap()[r0:r0 + 128, i0:i0 + XCH])
                    z0 = zstage.tile([128, XCH], mybir.dt.int16, tag="z0")
                    nc.scalar.activation(z0[:], xt[:], AF.Copy,
                                         bias=0.0, scale=float(inv_s))
                    z1 = zstage.tile([128, XCH], mybir.dt.int16, tag="z1")
                    nc.vector.tensor_scalar(z1[:], z0[:], 127.0, -127.0,
                                            ALU.min, ALU.max)
                    nc.scalar.dma_start(z_d.ap()[r0:r0 + 128, i0:i0 + XCH],
                                        z1[:])

            def emit_convert(q):
                """Transpose quarter q k-major and split into exact fp8
                h16/l tiles. Returns (h16, l) tiles [128, KT, Q]."""
                t0 = q * Q
                h16_t = hl_pool.tile([128, KT, Q], mybir.dt.float8e4,
                                     tag="h16")
                l_t = hl_pool.tile([128, KT, Q], mybir.dt.float8e4, tag="l")
                for k in range(KT):
                    zT = ztp.tile([128, Q], mybir.dt.int16, tag="zT")
                    nc.sync.dma_start_transpose(
                        zT[:], z_d.ap()[t0:t0 + Q, k * 128:(k + 1) * 128])
                    h = hp.tile([128, Q], mybir.dt.int16, tag="h")
                    nc.scalar.activation(h[:], zT[:], AF.Copy,
                                         bias=0.0, scale=0.0625)
                    nc.vector.tensor_scalar(h16_t[:, k, :], h[:], 16.0, None,
                                            ALU.mult)
                    nc.gpsimd.tensor_tensor(l_t[:, k, :], zT[:],
                                            h16_t[:, k, :], ALU.subtract)
                return h16_t, l_t

            for _rep in range(reps):
                for m in range(T // 128):
                    emit_quant(m)
                hls = [emit_convert(q) for q in range(NQ)]

                for n in range(NT):
                    wt = wt_pool.tile([128, KT, N_CHUNK], mybir.dt.float8e4,
                                      tag="wt")
                    nc.scalar.dma_start(wt[:], wt_d.ap()[:, n, :, :])
                    for q in range(NQ):
                        h16_t, l_t = hls[q]
                        for mm in range(MT):
                            ms = slice(mm * 128, (mm + 1) * 128)
                            psum = psum_pool.tile([128, N_CHUNK],
                                                  mybir.dt.float32)
                            for kk in range(KP):
                                ks = slice(2 * kk, 2 * kk + 2)
                                nc.tensor.matmul(
                                    psum[:], h16_t[:, ks, ms], wt[:, ks, :],
                                    start=(kk == 0), stop=False,
                                    perf_mode=DR)
                            for kk in range(KP):
                                ks = slice(2 * kk, 2 * kk + 2)
                                nc.tensor.matmul(
                                    psum[:], l_t[:, ks, ms], wt[:, ks, :],
                                    start=False, stop=(kk == KP - 1),
                                    perf_mode=DR)
                            osb = out_pool.tile([128, N_CHUNK],
                                                mybir.dt.bfloat16, tag="osb")
                            nc.vector.scalar_tensor_tensor(
                                osb[:], psum[:], float(alpha_s),
                                bias_t[:, n * N_CHUNK:(n + 1) * N_CHUNK],
                                ALU.mult, ALU.add)
                            t0 = q * Q + mm * 128
                            nc.scalar.dma_start(
                                out_d.ap()[t0:t0 + 128,
                                           n * N_CHUNK:(n + 1) * N_CHUNK],
                                osb[:])

    nc.compile()
    return nc


def prep_scalars(alpha, act_scale):
    s = max(float(np.asarray(act_scale)), 1e-5)
    inv_s = 1.0 / np.float32(s)
    alpha_s = float(np.float32(np.asarray(alpha, dtype=np.float32)) *
                    np.float32(s))
    return float(inv_s), alpha_s


def prep_weights(packed_w, bias):
    """Host-side weight/bias packing (replicated across cores)."""
    w_sign = np.asarray(packed_w, dtype=np.float32) - 1.0     # [OUT, IN]
    # wt[p, n, k, c] = w_sign[n*512 + c, k*128 + p]
    wt = w_sign.reshape(NT, N_CHUNK, KT, 128).transpose(3, 0, 2, 1)
    whost = np.ascontiguousarray(wt.astype(ml_dtypes.float8_e4m3))
    bias_rep = np.ascontiguousarray(
        np.broadcast_to(
            np.asarray(bias, dtype=np.float32).astype(ml_dtypes.bfloat16)
            [None, :], (128, OUT)))
    return whost, bias_rep


def kernel(x, packed_w, alpha, act_scale, bias, _trace=False):
    from concourse.bass_utils import run_bass_kernel_spmd

    x2d = np.asarray(x, dtype=np.float32).reshape(TOKENS, IN)
    inv_s, alpha_s = prep_scalars(alpha, act_scale)
    whost, bias_rep = prep_weights(packed_w, bias)

    nc = _build_program(inv_s, alpha_s)

    in_maps = [
        {"x": np.ascontiguousarray(x2d[c * T:(c + 1) * T]),
         "wt": whost, "bias": bias_rep}
        for c in range(N_CORES)
    ]
    res = run_bass_kernel_spmd(nc, in_maps, list(range(N_CORES)),
                               trace=_trace)

    out = np.empty((TOKENS, OUT), dtype=np.float32)
    for c in range(N_CORES):
        out[c * T:(c + 1) * T] = np.asarray(res.results[c]["out"],
                                            dtype=np.float32)
    out = out.reshape(B, S, OUT)
    if _trace:
        return out, res
    return out


# revision 5
# speedup vs baseline: 2.8845x; 2.2190x over previous
"""BitLinear (ternary-weight linear) Trainium2 kernel — fp8 DoubleRow version.

Math (matching the reference):
    s      = max(act_scale, 1e-5)
    z      = clip(round(x / s), -127, 127)           # int8-valued
    out    = (alpha * s) * (z @ sign(W).T) + bias

Key idea: TRN2's fp8 DoubleRow matmul contracts 2 k-tiles (256 deep) per
instruction at 0.5 cycles per output row -> 4x bf16 MAC throughput. z in
[-127,127] is not exact in fp8e4 (4 sig bits), so split exactly:
    h   = round(z / 16)        (any rounding mode works)
    h16 = 16 * h               in {-128..128, step 16}  -> exact in fp8e4
    l   = z - h16              in [-15, 15] integers    -> exact in fp8e4
    z @ W = h16 @ W + l @ W    (both passes accumulate into the same PSUM
                                bank; f32 accumulator keeps integer
                                arithmetic exact)
Two DoubleRow passes = 2x bf16 matmul throughput overall: ~437us of PE
time per core vs the bf16 baseline's ~874us.

Device strategy (8 cores, data-parallel over the 16384 tokens, 2048 each):
  - Quantize x token-major (ACT round+scale, DVE clamp) -> z int16, bounce
    through DRAM, xbar-DMA-transpose (2-byte granularity) back k-major.
  - Convert zT -> h16/l fp8 tiles on ACT/DVE/Pool (layout- and
    rounding-mode-robust: any h with |z-16h|<=15 recombines exactly).
  - Weights replicated, host-packed to fp8e4 [128, NT, KT, 512]; streamed
    once (16MB) as one n-chunk at a time, double buffered.
  - PSUM drain fused on DVE: out = psum * (alpha*s) + bias -> bf16 store
    (bf16 output rounding ~1.3e-3 rel, well within 2e-2; host upcasts).
"""

import sys

sys.path.insert(0, "/opt/trn_rl_repo")

import numpy as np
import ml_dtypes

# ---- problem constants (hardcoded per harness contract) ----
B, S, IN, OUT = 4, 4096, 4096, 4096
TOKENS = B * S              # 16384
N_CORES = 8
T = TOKENS // N_CORES       # 2048 tokens per core
KT = IN // 128              # 32 k-tiles (contraction)
KP = KT // 2                # 16 DoubleRow k-pairs
N_CHUNK = 512               # output columns per PSUM tile
NT = OUT // N_CHUNK         # 8 n-chunks
Q = 512                     # token-quarter (transpose/convert granularity)
NQ = T // Q                 # 4 quarters
MT = Q // 128               # 4 m-tiles per quarter
XCH = 1024                  # free-dim chunk for quantization staging


def _build_program(inv_s: float, alpha_s: float, reps: int = 1):
    import concourse.mybir as mybir
    import concourse.tile as tile
    from concourse import bacc

    nc = bacc.Bacc("TRN2", target_bir_lowering=False, debug=False,
                   num_devices=N_CORES)

    x_d = nc.dram_tensor("x", [T, IN], mybir.dt.float32, kind="ExternalInput")
    # wt[p, n, k, c] = sign(W)[n*512 + c, k*128 + p]
    wt_d = nc.dram_tensor("wt", [128, NT, KT, N_CHUNK], mybir.dt.float8e4,
                          kind="ExternalInput")
    bias_d = nc.dram_tensor("bias", [128, OUT], mybir.dt.bfloat16,
                            kind="ExternalInput")
    out_d = nc.dram_tensor("out", [T, OUT], mybir.dt.bfloat16,
                           kind="ExternalOutput")
    z_d = nc.dram_tensor("z_scratch", [T, IN], mybir.dt.int16)

    AF = mybir.ActivationFunctionType
    ALU = mybir.AluOpType
    DR = mybir.MatmulPerfMode.DoubleRow

    with tile.TileContext(nc) as tc:
        with (
            tc.tile_pool(name="xstage", bufs=2) as xstage,
            tc.tile_pool(name="zstage", bufs=2) as zstage,
            tc.tile_pool(name="ztp", bufs=4) as ztp,
            tc.tile_pool(name="hp", bufs=4) as hp,
            tc.tile_pool(name="hl", bufs=NQ) as hl_pool,
            tc.tile_pool(name="wtp", bufs=2) as wt_pool,
            tc.tile_pool(name="outsb", bufs=4) as out_pool,
            tc.tile_pool(name="biasp", bufs=1) as bias_pool,
            tc.tile_pool(name="psum", bufs=8, space="PSUM") as psum_pool,
        ):
            bias_t = bias_pool.tile([128, OUT], mybir.dt.bfloat16, tag="bias")
            nc.scalar.dma_start(bias_t[:], bias_d.ap())

            # DMA ring assignment: SP (nc.sync) carries the xbar transposes;
            # Pool/SWDGE (nc.gpsimd) carries x loads; ACT (nc.scalar)
            # carries z stores, weight loads, bias load and output stores.
            def emit_quant(m):
                """Quantize one 128-token row block: x -> round/clip ->
                int16, bounce to DRAM."""
                r0 = m * 128
                for c in range(IN // XCH):
                    i0 = c * XCH
                    xt = xstage.tile([128, XCH], mybir.dt.float32, tag="xf32")
                    nc.gpsimd.dma_start(xt[:],
                                        x_d.ap()[r0:r0 + 128, i0:i0 + XCH])
                    z0 = zstage.tile([128, XCH], mybir.dt.int16, tag="z0")
                    nc.scalar.activation(z0[:], xt[:], AF.Copy,
                                         bias=0.0, scale=float(inv_s))
                    z1 = zstage.tile([128, XCH], mybir.dt.int16, tag="z1")
                    nc.vector.tensor_scalar(z1[:], z0[:], 127.0, -127.0,
                                            ALU.min, ALU.max)
                    nc.scalar.dma_start(z_d.ap()[r0:r0 + 128, i0:i0 + XCH],
                                        z1[:])

            def emit_convert(q):
                """Transpose quarter q k-major and split into exact fp8
                h16/l tiles. Returns (h16, l) tiles [128, KT, Q]."""
                t0 = q * Q
                h16_t = hl_pool.tile([128, KT, Q], mybir.dt.float8e4,
                                     tag="h16")
                l_t = hl_pool.tile([128, KT, Q], mybir.dt.float8e4, tag="l")
                for k in range(KT):
                    zT = ztp.tile([128, Q], mybir.dt.int16, tag="zT")
                    nc.sync.dma_start_transpose(
                        zT[:], z_d.ap()[t0:t0 + Q, k * 128:(k + 1) * 128])
                    h = hp.tile([128, Q], mybir.dt.int16, tag="h")
                    nc.scalar.activation(h[:], zT[:], AF.Copy,
                                         bias=0.0, scale=0.0625)
                    nc.vector.tensor_scalar(h16_t[:, k, :], h[:], 16.0, None,
                                            ALU.mult)
                    nc.gpsimd.tensor_tensor(l_t[:, k, :], zT[:],
                                            h16_t[:, k, :], ALU.subtract)
                return h16_t, l_t

            for _rep in range(reps):
                hls = []
                for q in range(NQ):
                    for mm in range(MT):
                        emit_quant(q * MT + mm)
                    hls.append(emit_convert(q))

                for n in range(NT):
                    wt = wt_pool.tile([128, KT, N_CHUNK], mybir.dt.float8e4,
                                      tag="wt")
                    nc.scalar.dma_start(wt[:], wt_d.ap()[:, n, :, :])
                    for q in range(NQ):
                        h16_t, l_t = hls[q]
                        for mm in range(MT):
                            ms = slice(mm * 128, (mm + 1) * 128)
                            psum = psum_pool.tile([128, N_CHUNK],
                                                  mybir.dt.float32)
                            for kk in range(KP):
                                ks = slice(2 * kk, 2 * kk + 2)
                                nc.tensor.matmul(
                                    psum[:], h16_t[:, ks, ms], wt[:, ks, :],
                                    start=(kk == 0), stop=False,
                                    perf_mode=DR)
                            for kk in range(KP):
                                ks = slice(2 * kk, 2 * kk + 2)
                                nc.tensor.matmul(
                                    psum[:], l_t[:, ks, ms], wt[:, ks, :],
                                    start=False, stop=(kk == KP - 1),
                                    perf_mode=DR)
                            osb = out_pool.tile([128, N_CHUNK],
                                                mybir.dt.bfloat16, tag="osb")
                            nc.vector.scalar_tensor_tensor(
                                osb[:], psum[:], float(alpha_s),
                                bias_t[:, n * N_CHUNK:(n + 1) * N_CHUNK],
                                ALU.mult, ALU.add)
                            t0 = q * Q + mm * 128
                            nc.scalar.dma_start(
                                out_d.ap()[t0:t0 + 128,
                                           n * N_CHUNK:(n + 1) * N_CHUNK],
                                osb[:])

    nc.compile()
    return nc


def prep_scalars(alpha, act_scale):
    s = max(float(np.asarray(act_scale)), 1e-5)
    inv_s = 1.0 / np.float32(s)
    alpha_s = float(np.float32(np.asarray(alpha, dtype=np.float32)) *
                    np.float32(s))
    return float(inv_s), alpha_s


def prep_weights(packed_w, bias):
    """Host-side weight/bias packing (replicated across cores)."""
    w_sign = np.asarray(packed_w, dtype=np.float32) - 1.0     # [OUT, IN]
    # wt[p, n, k, c] = w_sign[n*512 + c, k*128 + p]
    wt = w_sign.reshape(NT, N_CHUNK, KT, 128).transpose(3, 0, 2, 1)
    whost = np.ascontiguousarray(wt.astype(ml_dtypes.float8_e4m3))
    bias_rep = np.ascontiguousarray(
        np.broadcast_to(
            np.asarray(bias, dtype=np.float32).astype(ml_dtypes.bfloat16)
            [None, :], (128, OUT)))
    return whost, bias_rep


def kernel(x, packed_w, alpha, act_scale, bias, _trace=False):
    from concourse.bass_utils import run_bass_kernel_spmd

    x2d = np.asarray(x, dtype=np.float32).reshape(TOKENS, IN)
    inv_s, alpha_s = prep_scalars(alpha, act_scale)
    whost, bias_rep = prep_weights(packed_w, bias)

    nc = _build_program(inv_s, alpha_s)

    in_maps = [
        {"x": np.ascontiguousarray(x2d[c * T:(c + 1) * T]),
         "wt": whost, "bias": bias_rep}
        for c in range(N_CORES)
    ]
    res = run_bass_kernel_spmd(nc, in_maps, list(range(N_CORES)),
                               trace=_trace)

    out = np.empty((TOKENS, OUT), dtype=np.float32)
    for c in range(N_CORES):
        out[c * T:(c + 1) * T] = np.asarray(res.results[c]["out"],
                                            dtype=np.float32)
    out = out.reshape(B, S, OUT)
    if _trace:
        return out, res
    return out
